# revision 1
# baseline (speedup 1.0000x reference)
"""Arcee decoder layer on 8 TRN2 NeuronCores — tensor-parallel Bass kernel.

Sharding (8-way TP, transposed activation layout [hidden, seq] on device):
  - core c owns: q heads 4c..4c+3 + kv head c (GQA group), residual-stream
    rows 512c..512c+512, intermediate cols 2048c..2048c+2048.
  - RMSNorm trick: the un-normalized residual stream is AllGathered (bf16)
    with each core's partial sum-of-squares embedded as an extra row; every
    core derives the per-token rsqrt scale locally and the scale is folded
    into the next matmul's PSUM eviction (norm scale commutes with the
    matmul). ln weights are folded into the bf16 weight caches.
  - o_proj / down_proj emit transposed partials [4096,S]; bf16 ReduceScatter
    gives each core its hid-slice of the sum = its slice of the transposed
    outputs. Host reassembles by concat + transpose.
  - dtypes: all matmuls bf16 (f32 PSUM accumulation); residual stream and
    softmax statistics f32.
"""
import sys

sys.path.insert(0, "/opt/trn_rl_repo")

import math
import numpy as np

import concourse.bass as bass
import concourse.mybir as mybir
import concourse.tile as tile
from concourse import bacc
from concourse.bass_isa import ReduceOp
from concourse.masks import make_identity

F32 = mybir.dt.float32
BF16 = mybir.dt.bfloat16
I32 = mybir.dt.int32
AF = mybir.ActivationFunctionType
ALU = mybir.AluOpType

N_CORES = 8
S = 2048
HID = 4096
N_HEADS = 32
N_KV = 8
DHEAD = 128
INTER = 16384
EPS = 1e-5
THETA = 10000.0

HQ = N_HEADS // N_CORES          # 4 q heads per core
HID_SH = HID // N_CORES          # 512 residual rows per core
INT_SH = INTER // N_CORES        # 2048 intermediate per core
NJ = HQ + 2                      # qkv col tiles per core (4q + k + v)
QKV_COLS = NJ * DHEAD            # 768
P = 128
SC = 512                         # seq chunk (matmul moving dim)
NSC = S // SC                    # 4
NT_HID = HID // P                # 32
NT_HSH = HID_SH // P             # 4
NT_INT = INT_SH // P             # 16
BLK = HID_SH + 8                 # AG block rows: 512 payload + ssq row + pad
TWO_PI = 2.0 * math.pi


def build_graph():
    nc = bacc.Bacc(None, target_bir_lowering=False, debug=False)

    hT = nc.declare_dram_parameter("hT", [HID_SH, S], F32, isOutput=False)
    rT = nc.declare_dram_parameter("rT", [HID_SH, S], F32, isOutput=False)
    pos_in = nc.declare_dram_parameter("positions", [1, S], I32, isOutput=False)
    wqkv = nc.declare_dram_parameter("wqkv", [HID, QKV_COLS], F32, isOutput=False)
    wo = nc.declare_dram_parameter("wo", [HQ * DHEAD, HID], F32, isOutput=False)
    wup = nc.declare_dram_parameter("wup", [HID, INT_SH], F32, isOutput=False)
    wdn = nc.declare_dram_parameter("wdn", [INT_SH, HID], F32, isOutput=False)
    ln1 = nc.declare_dram_parameter("ln1", [P, NT_HID], F32, isOutput=False)
    ln2 = nc.declare_dram_parameter("ln2", [P, NT_HID], F32, isOutput=False)
    ln2s = nc.declare_dram_parameter("ln2s", [P, NT_HSH], F32, isOutput=False)
    out_res2 = nc.declare_dram_parameter("res2T", [HID_SH, S], F32, isOutput=True)
    out_mlp = nc.declare_dram_parameter("mlpT", [HID_SH, S], F32, isOutput=True)

    RG = [list(range(N_CORES))]
    inv_sqrt_d = 1.0 / math.sqrt(DHEAD)

    with tile.TileContext(nc) as tc:
        import contextlib
        with contextlib.ExitStack() as ctx:
            const = ctx.enter_context(tc.tile_pool(name="const", bufs=1))
            rowsb = ctx.enter_context(tc.tile_pool(name="rowsb", bufs=1))
            acc = ctx.enter_context(tc.tile_pool(name="acc", bufs=6, space="PSUM"))
            dram = ctx.enter_context(tc.tile_pool(name="dram", bufs=1, space="DRAM"))

            # ============ constants ============
            ident = const.tile([P, P], BF16)
            make_identity(nc, ident[:])
            ones_bf = const.tile([P, 1], BF16)
            nc.vector.memset(ones_bf[:], 1.0)
            ln1_sb = const.tile([P, NT_HID], F32)
            ln2s_sb = const.tile([P, NT_HSH], F32)
            nc.sync.dma_start(ln1_sb[:], ln1[:])
            nc.sync.dma_start(ln2s_sb[:], ln2s[:])
            cos2 = const.tile([P, S], BF16)
            sin_neg = const.tile([P, S], BF16)
            # causal masks for the 4 diagonal sk-tile offsets within a chunk:
            # mask[j]: keep (=1.0) where f - p - 128*j >= 0 else 0
            cmask = []
            for j in range(SC // P):
                mk = const.tile([P, SC], BF16, name=f"cmask{j}")
                nc.vector.memset(mk[:], 1.0)
                nc.gpsimd.affine_select(mk[:], mk[:], pattern=[[1, SC]],
                                        base=-j * P, channel_multiplier=-1,
                                        compare_op=ALU.is_ge, fill=0.0)
                cmask.append(mk)

            # DRAM scratch
            wo_c = dram.tile([P, HQ * HID], BF16, name="wo_c")
            SH = S // 2
            ag1_in = [dram.tile([BLK, SH], BF16, name=f"ag1_in{h}") for h in range(2)]
            ag1_out = [dram.tile([N_CORES * BLK, SH], BF16, name=f"ag1_out{h}",
                                 addr_space="Shared") for h in range(2)]
            ag2_in = [dram.tile([BLK, SH], BF16, name=f"ag2_in{h}") for h in range(2)]
            ag2_out = [dram.tile([N_CORES * BLK, SH], BF16, name=f"ag2_out{h}",
                                 addr_space="Shared") for h in range(2)]
            rs1_in = [dram.tile([HID, SC], BF16, name=f"rs1_in{sc}") for sc in range(NSC)]
            rs1_out = [dram.tile([HID_SH, SC], BF16, name=f"rs1_out{sc}")
                       for sc in range(NSC)]
            NQ = 4
            QROWS = HID // NQ                  # 1024 input rows per quarter
            QOUT = QROWS // N_CORES            # 128 output rows per quarter
            rs2_in = [dram.tile([QROWS, S], BF16, name=f"rs2_in{q}")
                      for q in range(NQ)]
            rs2_out = [dram.tile([QOUT, S], BF16, name=f"rs2_out{q}")
                       for q in range(NQ)]

            ag1_v = [t[:].rearrange("(c r) s -> c r s", r=BLK) for t in ag1_out]
            ag2_v = [t[:].rearrange("(c r) s -> c r s", r=BLK) for t in ag2_out]

            def half_cs(sc):
                return sc // 2, slice((sc % 2) * SC, (sc % 2 + 1) * SC)
            wo_v = wo_c[:].rearrange("p (a c) -> p a c", a=HQ)

            bcd = [dram.tile([1, SC], F32, name=f"bcd{i}") for i in range(8)]
            bcdb = [dram.tile([1, SC], BF16, name=f"bcdb{i}") for i in range(8)]
            _bci = [0]

            def row_broadcast(dst_ap, src_row):
                # dst[P, SC] <- broadcast of src_row[1, SC] via DRAM bounce;
                # bf16 dst goes through a bf16 bounce (DMA cannot cast)
                i = _bci[0] % len(bcd)
                _bci[0] += 1
                if dst_ap.dtype == BF16:
                    d = bcdb[i]
                else:
                    d = bcd[i]
                nc.sync.dma_start(d[:], src_row)
                nc.sync.dma_start(dst_ap, d[:].to_broadcast((P, SC)))

            def hg_src(ag_v, k, cs):
                # global hid tile k of the gathered (blocked) activation
                return ag_v[k // NT_HSH, (k % NT_HSH) * P:(k % NT_HSH + 1) * P, cs]

            # ============ rope tables (scoped scratch) ============
            with tc.tile_pool(name="tbl", bufs=1) as tbl:
                iot = tbl.tile([64, 1], I32)
                nc.gpsimd.iota(iot[:], pattern=[[1, 1]], base=0, channel_multiplier=1)
                iotf = tbl.tile([64, 1], F32)
                nc.vector.tensor_copy(iotf[:], iot[:])
                invf = tbl.tile([64, 1], F32)
                nc.scalar.activation(invf[:], iotf[:], AF.Exp,
                                     scale=-math.log(THETA) / 64.0)
                invf2pi = tbl.tile([64, 1], F32)
                nc.scalar.activation(invf2pi[:], invf[:], AF.Copy,
                                     scale=1.0 / TWO_PI)
                posi = tbl.tile([1, S], I32)
                nc.sync.dma_start(posi[:], pos_in[:])
                posf = tbl.tile([1, S], F32)
                nc.vector.tensor_copy(posf[:], posi[:])
                posb = tbl.tile([64, S], F32)
                nc.gpsimd.partition_broadcast(posb[:], posf[:])

                def range_reduce_sin(dst_bf, t_ap, negate=False):
                    # dst = sin(2*pi*t) via two-stage round-and-subtract
                    n1 = tbl.tile([64, S], I32, tag="rri", bufs=2, name="n1")
                    nc.vector.tensor_copy(n1[:], t_ap)
                    n1f = tbl.tile([64, S], F32, tag="rrf", bufs=2, name="n1f")
                    nc.vector.tensor_copy(n1f[:], n1[:])
                    f1 = tbl.tile([64, S], F32, tag="rrg", bufs=2, name="f1")
                    nc.vector.tensor_tensor(f1[:], t_ap, n1f[:], ALU.subtract)
                    n2 = tbl.tile([64, S], I32, tag="rri", bufs=2, name="n2")
                    nc.vector.tensor_copy(n2[:], f1[:])
                    n2f = tbl.tile([64, S], F32, tag="rrf", bufs=2, name="n2f")
                    nc.vector.tensor_copy(n2f[:], n2[:])
                    f2 = tbl.tile([64, S], F32, tag="rrg", bufs=2, name="f2")
                    nc.vector.tensor_tensor(f2[:], f1[:], n2f[:], ALU.subtract)
                    nc.scalar.activation(dst_bf, f2[:], AF.Sin,
                                         scale=-TWO_PI if negate else TWO_PI)

                tfrac = tbl.tile([64, S], F32)
                nc.scalar.activation(tfrac[:], posb[:], AF.Copy, scale=invf2pi[:])
                sinb = tbl.tile([64, S], BF16)
                sinnb = tbl.tile([64, S], BF16)
                range_reduce_sin(sinb[:], tfrac[:])
                range_reduce_sin(sinnb[:], tfrac[:], negate=True)
                tfrac2 = tbl.tile([64, S], F32)
                nc.scalar.activation(tfrac2[:], tfrac[:], AF.Copy, bias=0.25)
                cosb = tbl.tile([64, S], BF16)
                range_reduce_sin(cosb[:], tfrac2[:])
                nc.sync.dma_start(cos2[:64, :], cosb[:])
                nc.sync.dma_start(cos2[64:, :], cosb[:])
                nc.sync.dma_start(sin_neg[:64, :], sinnb[:])
                nc.sync.dma_start(sin_neg[64:, :], sinb[:])

            # ================== attention era ==================
            with tc.tile_pool(name="apersist", bufs=1) as apersist, \
                 tc.tile_pool(name="awork", bufs=1) as awork, \
                 tc.tile_pool(name="wstr", bufs=1) as wstr, \
                 tc.tile_pool(name="rowps", bufs=1, space="PSUM") as rowps, \
                 tc.tile_pool(name="tpps", bufs=1, space="PSUM") as tpps:

                _cnt = [0]

                def t2k(tag="t2k", bufs=7):
                    _cnt[0] += 1
                    return awork.tile([P, SC], F32, tag=tag, bufs=bufs,
                                      name=f"t_{_cnt[0]}")

                def t1k(tag="t1k", bufs=7):
                    _cnt[0] += 1
                    return awork.tile([P, SC], BF16, tag=tag, bufs=bufs,
                                      name=f"t_{_cnt[0]}")

                # ---- phase 1: x = h + r (chunked), ssq row, ag1_in (bf16)
                ssq1 = awork.tile([1, S], F32, name="ssq1")
                for sc in range(NSC):
                    cs = slice(sc * SC, (sc + 1) * SC)
                    ps = rowps.tile([1, SC], F32, tag="row", name=f"ssq1p{sc}")
                    for i in range(NT_HSH):
                        a = t2k()
                        b = t2k()
                        nc.sync.dma_start(a[:], hT[i * P:(i + 1) * P, cs])
                        nc.sync.dma_start(b[:], rT[i * P:(i + 1) * P, cs])
                        xt = t2k()
                        nc.vector.tensor_tensor(xt[:], a[:], b[:], ALU.add)
                        xb = t1k()
                        nc.vector.tensor_copy(xb[:], xt[:])
                        hh, hcs = half_cs(sc)
                        nc.sync.dma_start(ag1_in[hh][i * P:(i + 1) * P, hcs], xb[:])
                        sq = t1k(tag="sq", bufs=2)
                        nc.scalar.activation(sq[:], xt[:], AF.Square)
                        nc.tensor.matmul(ps[:], ones_bf[:], sq[:],
                                         start=(i == 0), stop=(i == NT_HSH - 1))
                    nc.vector.tensor_copy(ssq1[:, cs], ps[:])
                    if sc % 2 == 1:
                        hh = sc // 2
                        ssq1b = awork.tile([1, SH], BF16, tag="ssq1b", bufs=2,
                                           name=f"ssq1b{hh}")
                        nc.vector.tensor_copy(ssq1b[:],
                                              ssq1[:, hh * SH:(hh + 1) * SH])
                        nc.sync.dma_start(ag1_in[hh][HID_SH:HID_SH + 1, :], ssq1b[:])
                        nc.gpsimd.collective_compute(
                            "AllGather", ALU.bypass, replica_groups=RG,
                            ins=[ag1_in[hh][:].opt()], outs=[ag1_out[hh][:].opt()])

                # ---- weight caches (emitted after AG1 so phase-1 DMAs win)
                def build_cache(src, n_row_tiles, n_cols, dst, ln_col, eng):
                    CB = min(n_cols, 768)
                    for k in range(n_row_tiles):
                        for c0 in range(0, n_cols, CB):
                            wf = wstr.tile([P, CB], F32, tag="cbf", bufs=2,
                                           name=f"cb_{dst.tensor.name}_{k}_{c0}")
                            nc.scalar.dma_start(wf[:, :min(CB, n_cols - c0)],
                                                src[k * P:(k + 1) * P,
                                                    c0:c0 + min(CB, n_cols - c0)])
                            wb = wstr.tile([P, CB], BF16, tag="cbb", bufs=2,
                                           name=f"cc_{dst.tensor.name}_{k}_{c0}")
                            w = min(CB, n_cols - c0)
                            if ln_col is not None:
                                nc.scalar.activation(wb[:, :w], wf[:, :w], AF.Copy,
                                                     scale=ln_col[:, k:k + 1])
                            else:
                                eng.tensor_copy(wb[:, :w], wf[:, :w])
                            nc.scalar.dma_start(
                                dst[:, k * n_cols + c0:k * n_cols + c0 + w],
                                wb[:, :w])

                # wqkv: convert once straight into persistent SBUF (ln1 folded)
                wqkv_sb = [apersist.tile([P, NT_HID, P], BF16, name=f"wqsb{j}")
                           for j in range(NJ)]
                for k in range(NT_HID):
                    wfq = wstr.tile([P, QKV_COLS], F32, tag="cbf", bufs=2,
                                    name=f"wfq{k}")
                    nc.scalar.dma_start(wfq[:], wqkv[k * P:(k + 1) * P, :])
                    for j in range(NJ):
                        nc.scalar.activation(wqkv_sb[j][:, k, :],
                                             wfq[:, j * P:(j + 1) * P], AF.Copy,
                                             scale=ln1_sb[:, k:k + 1])
                build_cache(wo, HQ, HID, wo_c, None, nc.vector)

                # per-chunk norm1 scale rows (from gathered ssq partials)
                s1b = apersist.tile([P, S], BF16, name="s1b")  # rsqrt scale bcast
                for sc in range(NSC):
                    cs = slice(sc * SC, (sc + 1) * SC)
                    hh, hcs = half_cs(sc)
                    srows_b = awork.tile([8, SC], BF16, tag="srb", bufs=1,
                                         name=f"sr1b{sc}")
                    nc.gpsimd.dma_start(srows_b[:], ag1_v[hh][:, HID_SH, hcs])
                    srows = awork.tile([8, SC], F32, tag="srf", bufs=1,
                                       name=f"sr1f{sc}")
                    nc.vector.tensor_copy(srows[:], srows_b[:])
                    ssum = awork.tile([8, SC], F32, tag="ssum", bufs=1,
                                      name=f"ss1{sc}")
                    nc.gpsimd.partition_all_reduce(ssum[:], srows[:], channels=8,
                                                   reduce_op=ReduceOp.add)
                    var = awork.tile([1, SC], F32, tag="var", bufs=2, name=f"v1{sc}")
                    nc.scalar.activation(var[:], ssum[:1, :], AF.Copy,
                                         scale=1.0 / HID, bias=EPS)
                    nc.vector.reciprocal(var[:], var[:])
                    varb = awork.tile([1, SC], BF16, tag="varb", bufs=2,
                                      name=f"v1b{sc}")
                    nc.scalar.activation(varb[:], var[:], AF.Sqrt)
                    row_broadcast(s1b[:, cs], varb[:])

                # k and v keep full-S persistent tiles; q tiles rotate per chunk
                kT = apersist.tile([P, S], BF16, name="kT")
                vT = apersist.tile([P, S], BF16, name="vT")

                qcs = {}

                def emit_qkv(sc):
                    cs = slice(sc * SC, (sc + 1) * SC)
                    cs = slice(sc * SC, (sc + 1) * SC)
                    # ---- qkv chunk (scale1 folded into eviction)
                    hgb = []
                    hh, hcs = half_cs(sc)
                    for cb in range(N_CORES):
                        g = awork.tile([P, NT_HSH, SC], BF16, tag="hg", bufs=8,
                                       name=f"hg{cb}_{sc}")
                        nc.gpsimd.dma_start(
                            g[:], ag1_v[hh][cb, :HID_SH, hcs].rearrange(
                                "(t p) s -> p t s", p=P))
                        hgb.append(g)
                    hg = [hgb[k // NT_HSH][:, k % NT_HSH, :] for k in range(NT_HID)]
                    qc = {}
                    for j in range(NJ):
                        ps = acc.tile([P, SC], F32, tag="acc", name=f"qk{j}_{sc}")
                        for k in range(NT_HID):
                            nc.tensor.matmul(ps[:], wqkv_sb[j][:, k, :], hg[k],
                                             start=(k == 0), stop=(k == NT_HID - 1))
                        if j < HQ:
                            dst = awork.tile([P, SC], BF16, tag="qc", bufs=10,
                                             name=f"qc{j}_{sc}")
                            qc[j] = dst
                            nc.vector.tensor_tensor(dst[:], ps[:], s1b[:, cs],
                                                    ALU.mult)
                        else:
                            dst = kT if j == HQ else vT
                            nc.vector.tensor_tensor(dst[:, cs], ps[:], s1b[:, cs],
                                                    ALU.mult)

                    qcs[sc] = qc

                def emit_attn(sc):
                    cs = slice(sc * SC, (sc + 1) * SC)
                    qc = qcs[sc]
                    # ---- rope on q tiles and k tile (bf16, chunk cols)
                    for j in range(HQ + 1):
                        tv = qc[j][:] if j < HQ else kT[:, cs]
                        swp = t1k()
                        nc.sync.dma_start(swp[:64, :], tv[64:, :])
                        nc.sync.dma_start(swp[64:, :], tv[:64, :])
                        m1 = t1k()
                        nc.vector.tensor_tensor(m1[:], tv, cos2[:, cs], ALU.mult)
                        m2 = t1k()
                        nc.vector.tensor_tensor(m2[:], swp[:], sin_neg[:, cs], ALU.mult)
                        nc.vector.tensor_tensor(tv, m1[:], m2[:], ALU.add)

                    # ---- v transpose in place (block-transposed v)
                    for t in range(sc * (SC // P), (sc + 1) * (SC // P)):
                        pst = tpps.tile([P, P], BF16, tag="tp", name=f"tp{t}")
                        nc.tensor.transpose(pst[:], vT[:, t * P:(t + 1) * P],
                                            ident[:])
                        nc.vector.tensor_copy(vT[:, t * P:(t + 1) * P], pst[:])

                    # ---- attention (4 heads x this chunk)
                    nsk = (sc + 1) * (SC // P)
                    attnT = {}
                    for h in range(HQ):
                        pv = acc.tile([P, SC], F32, tag="acc", name=f"pv{h}_{sc}")
                        rs = rowps.tile([1, SC], F32, tag="row", name=f"rs{h}_{sc}")
                        for skt in range(nsk):
                            sps = acc.tile([P, SC], F32, tag="acc",
                                           name=f"s{h}_{sc}_{skt}")
                            nc.tensor.matmul(sps[:],
                                             kT[:, skt * P:(skt + 1) * P],
                                             qc[h][:], start=True, stop=True)
                            ex = t1k(tag="ex", bufs=5)
                            nc.scalar.activation(ex[:], sps[:], AF.Exp,
                                                 scale=inv_sqrt_d)
                            if skt >= 4 * sc:
                                nc.vector.tensor_tensor(ex[:], ex[:],
                                                        cmask[skt - 4 * sc][:],
                                                        ALU.mult)
                            nc.tensor.matmul(rs[:], ones_bf[:], ex[:],
                                             start=(skt == 0), stop=(skt == nsk - 1))
                            nc.tensor.matmul(pv[:],
                                             vT[:, skt * P:(skt + 1) * P],
                                             ex[:], start=(skt == 0),
                                             stop=(skt == nsk - 1))
                        rcp = awork.tile([1, SC], F32, tag="rcp", bufs=2,
                                         name=f"rcp{h}_{sc}")
                        nc.vector.reciprocal(rcp[:], rs[:])
                        rcpb = t2k(tag="rcpb", bufs=2)
                        row_broadcast(rcpb[:], rcp[:])
                        at = awork.tile([P, SC], BF16, tag="attnT", bufs=6,
                                        name=f"at{h}_{sc}")
                        nc.vector.tensor_tensor(at[:], pv[:], rcpb[:], ALU.mult)
                        attnT[h] = at

                    # ---- o_proj chunk -> bf16 ReduceScatter
                    for m in range(NT_HID):
                        wm = wstr.tile([P, HQ, P], BF16, tag="wos", bufs=2,
                                       name=f"wm{m}_{sc}")
                        nc.scalar.dma_start(wm[:], wo_v[:, :, m * P:(m + 1) * P])
                        ps = acc.tile([P, SC], F32, tag="acc", name=f"o{m}_{sc}")
                        for a in range(HQ):
                            nc.tensor.matmul(ps[:], wm[:, a, :], attnT[a][:],
                                             start=(a == 0), stop=(a == HQ - 1))
                        ev = t1k(tag="oev", bufs=2)
                        nc.vector.tensor_copy(ev[:], ps[:])
                        nc.sync.dma_start(rs1_in[sc][m * P:(m + 1) * P, :], ev[:])
                    nc.gpsimd.collective_compute(
                        "ReduceScatter", ALU.add, replica_groups=RG,
                        ins=[rs1_in[sc][:].opt()], outs=[rs1_out[sc][:].opt()])

                    # ---- residual2 chunk -> raw bf16 + ssq row into ag2_in
                    ps2 = rowps.tile([1, SC], F32, tag="row", name=f"ssq2_{sc}")
                    for i in range(NT_HSH):
                        o = t1k(tag="r2ld", bufs=2)
                        nc.gpsimd.dma_start(o[:], rs1_out[sc][i * P:(i + 1) * P, :])
                        hh, hcs = half_cs(sc)
                        xr = t1k(tag="xr", bufs=3)
                        nc.sync.dma_start(xr[:], ag1_in[hh][i * P:(i + 1) * P, hcs])
                        r2t = t2k(tag="r2", bufs=4)
                        nc.vector.tensor_tensor(r2t[:], o[:], xr[:], ALU.add)
                        nc.sync.dma_start(out_res2[i * P:(i + 1) * P, cs], r2t[:])
                        r2b = t1k()
                        nc.scalar.activation(r2b[:], r2t[:], AF.Copy,
                                             scale=ln2s_sb[:, i:i + 1])
                        hh, hcs = half_cs(sc)
                        nc.sync.dma_start(ag2_in[hh][i * P:(i + 1) * P, hcs], r2b[:])
                        sq = t1k(tag="sq", bufs=2)
                        nc.scalar.activation(sq[:], r2t[:], AF.Square)
                        nc.tensor.matmul(ps2[:], ones_bf[:], sq[:],
                                         start=(i == 0), stop=(i == NT_HSH - 1))
                    ssq2 = awork.tile([1, SC], BF16, tag="ssq2", bufs=2,
                                      name=f"sq2_{sc}")
                    nc.vector.tensor_copy(ssq2[:], ps2[:])
                    hh, hcs = half_cs(sc)
                    nc.sync.dma_start(ag2_in[hh][HID_SH:HID_SH + 1, hcs], ssq2[:])
                    if sc % 2 == 1:
                        nc.gpsimd.collective_compute(
                            "AllGather", ALU.bypass, replica_groups=RG,
                            ins=[ag2_in[hh][:].opt()], outs=[ag2_out[hh][:].opt()])


                emit_qkv(0)
                for sc in range(NSC):
                    if sc + 1 < NSC:
                        emit_qkv(sc + 1)
                    emit_attn(sc)

            # ================== MLP era ==================
            with tc.tile_pool(name="mpersist", bufs=1) as mpersist, \
                 tc.tile_pool(name="mwork", bufs=1) as mwork, \
                 tc.tile_pool(name="mstr", bufs=1) as mstr, \
                 tc.tile_pool(name="macc", bufs=2, space="PSUM") as macc:
                # per-chunk 1/var rows (scale2^2 fold for down eviction)
                s2b = mpersist.tile([P, S], F32, name="s2b")
                for sc in range(NSC):
                    cs = slice(sc * SC, (sc + 1) * SC)
                    hh, hcs = half_cs(sc)
                    srows_b = mwork.tile([8, SC], BF16, tag="srb", bufs=1,
                                         name=f"sr2b{sc}")
                    nc.gpsimd.dma_start(srows_b[:], ag2_v[hh][:, HID_SH, hcs])
                    srows = mwork.tile([8, SC], F32, tag="srf", bufs=1,
                                       name=f"sr2f{sc}")
                    nc.vector.tensor_copy(srows[:], srows_b[:])
                    ssum = mwork.tile([8, SC], F32, tag="ssum", bufs=1,
                                      name=f"ss2{sc}")
                    nc.gpsimd.partition_all_reduce(ssum[:], srows[:], channels=8,
                                                   reduce_op=ReduceOp.add)
                    var = mwork.tile([1, SC], F32, tag="var", bufs=2, name=f"v2{sc}")
                    nc.scalar.activation(var[:], ssum[:1, :], AF.Copy,
                                         scale=1.0 / HID, bias=EPS)
                    nc.vector.reciprocal(var[:], var[:])  # = scale2^2
                    row_broadcast(s2b[:, cs], var[:])

                uT = [mpersist.tile([P, S], BF16, name=f"uT{it}")
                      for it in range(NT_INT)]
                for half in range(2):
                    hs = [half * 2, half * 2 + 1]
                    h2gb = []
                    for cb in range(N_CORES):
                        g = mwork.tile([P, NT_HSH, 2 * SC], BF16, tag=f"h2g{cb}",
                                       bufs=1, name=f"h2g{cb}_{half}")
                        nc.sync.dma_start(
                            g[:], ag2_v[half][cb, :HID_SH, :].rearrange(
                                "(t p) s -> p t s", p=P))
                        h2gb.append(g)
                    h2g = [h2gb[k // NT_HSH][:, k % NT_HSH, :]
                           for k in range(NT_HID)]
                    wup_vv = wup[:].rearrange("(k p) c -> p k c", p=P)
                    for it in range(NT_INT):
                        KQ = NT_HID // 4
                        wts = []
                        for khh in range(4):
                            wfu = mstr.tile([P, KQ, P], F32, tag="wupf", bufs=2,
                                            name=f"wf{it}_{half}_{khh}")
                            nc.scalar.dma_start(
                                wfu[:], wup_vv[:, khh * KQ:(khh + 1) * KQ,
                                               it * P:(it + 1) * P])
                            wtb = mstr.tile([P, KQ, P], BF16, tag="wups", bufs=5,
                                            name=f"wt{it}_{half}_{khh}")
                            nc.scalar.activation(wtb[:], wfu[:], AF.Copy)
                            wts.append(wtb)
                        for ci, sc_ in enumerate(hs):
                            ps = acc.tile([P, SC], F32, tag="acc",
                                          name=f"up{it}_{sc_}")
                            for k in range(NT_HID):
                                wk = wts[k // KQ][:, k % KQ, :]
                                nc.tensor.matmul(ps[:], wk,
                                                 h2g[k][:, ci * SC:(ci + 1) * SC],
                                                 start=(k == 0),
                                                 stop=(k == NT_HID - 1)) \
                                    if False else \
                                    nc.tensor.matmul(ps[:], wk,
                                                     h2gb[k // NT_HSH][:, k % NT_HSH,
                                                                       ci * SC:(ci + 1) * SC],
                                                     start=(k == 0),
                                                     stop=(k == NT_HID - 1))
                            rl = mwork.tile([P, SC], F32, tag="relu", bufs=2,
                                            name=f"rl{it}_{sc_}")
                            nc.scalar.activation(rl[:], ps[:], AF.Relu)
                            nc.vector.tensor_tensor(
                                uT[it][:, sc_ * SC:(sc_ + 1) * SC], rl[:], rl[:],
                                ALU.mult)

                wdn_v = wdn[:].rearrange("(t p) c -> p t c", p=P)
                MQ = NT_HID // 4  # m tiles per RS2 quarter
                for m in range(NT_HID):
                    wdn_t = []
                    IH = NT_INT // 2
                    for ih in range(2):
                        wf = mstr.tile([P, IH, P], F32, tag="wdnf", bufs=2,
                                       name=f"wf{m}_{ih}")
                        nc.scalar.dma_start(
                            wf[:], wdn_v[:, ih * IH:(ih + 1) * IH,
                                         m * P:(m + 1) * P])
                        wb = mstr.tile([P, IH, P], BF16, tag="wdnb", bufs=2,
                                       name=f"wb{m}_{ih}")
                        nc.scalar.activation(wb[:], wf[:], AF.Copy)
                        wdn_t.append(wb)
                    for sc in range(NSC):
                        cs = slice(sc * SC, (sc + 1) * SC)
                        ps = macc.tile([P, SC], F32, tag="macc", name=f"dn{m}_{sc}")
                        for it in range(NT_INT):
                            nc.tensor.matmul(ps[:], wdn_t[it // IH][:, it % IH, :],
                                             uT[it][:, cs],
                                             start=(it == 0), stop=(it == NT_INT - 1))
                        ev = mwork.tile([P, SC], BF16, tag="dnev", bufs=3,
                                        name=f"dev{m}_{sc}")
                        nc.vector.tensor_tensor(ev[:], ps[:], s2b[:, cs], ALU.mult)
                        q_, mq_ = divmod(m, MQ)
                        nc.sync.dma_start(rs2_in[q_][mq_ * P:(mq_ + 1) * P, cs],
                                          ev[:])
                    if (m + 1) % MQ == 0:
                        q = m // MQ
                        nc.gpsimd.collective_compute(
                            "ReduceScatter", ALU.add, replica_groups=RG,
                            ins=[rs2_in[q][:].opt()],
                            outs=[rs2_out[q][:].opt()])
                        # out_mlp rows [128q:128q+128] hold this core's quarter-q
                        # slice (global hid rows 1024q + 128*core); host remaps.
                        nc.gpsimd.dma_start(out_mlp[q * P:(q + 1) * P, :],
                                            rs2_out[q][:])

    nc.compile()
    return nc


def shard_inputs(positions, hidden_states, residual, qkv_w, o_w, up_w, down_w,
                 ln1_w, ln2_w):
    hTf = np.ascontiguousarray(np.asarray(hidden_states).reshape(S, HID).T)
    rTf = np.ascontiguousarray(np.asarray(residual).reshape(S, HID).T)
    pos = np.ascontiguousarray(np.asarray(positions).reshape(1, S))
    ln1_t = np.ascontiguousarray(np.asarray(ln1_w).reshape(NT_HID, P).T)  # [128,32]
    ln2_t = np.ascontiguousarray(np.asarray(ln2_w).reshape(NT_HID, P).T)
    q_size = N_HEADS * DHEAD
    kv = N_KV * DHEAD
    in_maps = []
    for c in range(N_CORES):
        wqkv_c = np.concatenate([
            qkv_w[:, c * HQ * DHEAD:(c + 1) * HQ * DHEAD],
            qkv_w[:, q_size + c * DHEAD:q_size + (c + 1) * DHEAD],
            qkv_w[:, q_size + kv + c * DHEAD:q_size + kv + (c + 1) * DHEAD],
        ], axis=1)
        in_maps.append({
            "hT": np.ascontiguousarray(hTf[c * HID_SH:(c + 1) * HID_SH]),
            "rT": np.ascontiguousarray(rTf[c * HID_SH:(c + 1) * HID_SH]),
            "positions": pos,
            "wqkv": np.ascontiguousarray(wqkv_c),
            "wo": np.ascontiguousarray(o_w[c * HQ * DHEAD:(c + 1) * HQ * DHEAD, :]),
            "wup": np.ascontiguousarray(up_w[:, c * INT_SH:(c + 1) * INT_SH]),
            "wdn": np.ascontiguousarray(down_w[c * INT_SH:(c + 1) * INT_SH, :]),
            "ln1": ln1_t,
            "ln2": ln2_t,
            "ln2s": np.ascontiguousarray(ln2_t[:, c * NT_HSH:(c + 1) * NT_HSH]),
        })
    return in_maps


_CACHE = {}


def kernel(**inputs):
    from concourse.bass_utils import run_bass_kernel_spmd
    if "nc" not in _CACHE:
        _CACHE["nc"] = build_graph()
    nc = _CACHE["nc"]
    in_maps = shard_inputs(**{k: np.asarray(v) for k, v in inputs.items()})
    res = run_bass_kernel_spmd(nc, in_maps, core_ids=list(range(N_CORES)),
                               trace=False)
    res2T = np.concatenate([res.results[c]["res2T"] for c in range(N_CORES)], axis=0)
    mlpT = np.empty((HID, S), np.float32)
    for c in range(N_CORES):
        mt = res.results[c]["mlpT"]
        for q in range(4):
            mlpT[q * 1024 + c * 128:q * 1024 + (c + 1) * 128] = \
                mt[q * 128:(q + 1) * 128]
    mlp_out = np.ascontiguousarray(mlpT.T).reshape(1, S, HID)
    residual2 = np.ascontiguousarray(res2T.T).reshape(1, S, HID)
    return mlp_out, residual2



# revision 14
# speedup vs baseline: 1.5111x; 1.5111x over previous
"""Arcee decoder layer on 8 TRN2 NeuronCores — TP8, fp8 hi/lo DoubleRow.

Sharding (8-way TP, transposed activation layout [hidden, seq] on device):
  - core c owns: q heads 4c..4c+3 + kv head c, residual rows 512c..512c+511,
    intermediate cols 2048c..2048c+2047.
  - Big GEMMs (qkv/o/up/down) run as 3-term hi/lo fp8e4m3 DoubleRow:
    W·x ~= Whi·xhi + Whi·xlo + Wlo·xhi, each term contracting 256 rows per
    0.5-cycle/row matmul. Weights pre-quantized on host (ln1/ln2 and
    per-tensor scales folded); activations split hi/lo on device.
  - RMSNorm: un-normalized residual stream AllGathered with per-core partial
    sum-of-squares row embedded; rsqrt scale folded into PSUM eviction.
  - AG payload [520, 1024] bf16 per S-half: rows 0-511 carry x rows as fp8
    bytes (hi in bf16 cols 0-511, lo in 512-1023), row 512 = bf16 ssq row.
  - o_proj / down_proj partials reduce via bf16 ReduceScatter; down RS split
    into uneven pieces so the exposed tail is small.
  - attention (scores/softmax/PV) stays bf16.
"""
import sys

sys.path.insert(0, "/opt/trn_rl_repo")

import contextlib
import math
import numpy as np
import ml_dtypes

import concourse.bass as bass
import concourse.mybir as mybir
import concourse.tile as tile
from concourse import bacc
from concourse.bass_isa import ReduceOp
from concourse.masks import make_identity

F32 = mybir.dt.float32
BF16 = mybir.dt.bfloat16
F8 = mybir.dt.float8e4
I32 = mybir.dt.int32
AF = mybir.ActivationFunctionType
ALU = mybir.AluOpType
DR = mybir.MatmulPerfMode.DoubleRow
F8NP = ml_dtypes.float8_e4m3

N_CORES = 8
S = 2048
HID = 4096
N_HEADS = 32
N_KV = 8
DHEAD = 128
INTER = 16384
EPS = 1e-5
THETA = 10000.0

HQ = N_HEADS // N_CORES          # 4 q heads per core
HID_SH = HID // N_CORES          # 512 residual rows per core
INT_SH = INTER // N_CORES        # 2048 intermediate per core
NJ = HQ + 2                      # qkv col tiles per core (4q + k + v)
QKV_COLS = NJ * DHEAD            # 768
P = 128
SC = 512                         # seq chunk
NSC = S // SC                    # 4
SH = S // 2                      # 1024 (half)
NT_HID = HID // P                # 32
NT_HSH = HID_SH // P             # 4
NT_INT = INT_SH // P             # 16
NPH = NT_HID // 2                # 16 k-pairs over HID
NPI = NT_INT // 2                # 8 k-pairs over INT_SH
BLK = HID_SH + 8                 # 520 payload rows
TWO_PI = 2.0 * math.pi

# fp8 scales (activations unscaled; weights scaled on host)
SW1 = 1024.0
SWO = 1024.0
SWU = 1024.0
SWD = 1024.0
SQU = 0.25                       # scale on u = relu(z)^2
CE1 = 1.0 / SW1                  # qkv evict const (with rsqrt row)
CO = 1.0 / SWO                   # o evict const
ALPHA_UP = math.sqrt(SQU) / SWU  # relu evict scale
CD = 1.0 / (SWD * SQU)           # down evict const (with 1/var row)

# down RS pieces (m-tile counts; sum = 32). last small => short tail.
PIECES = [8, 8, 8, 4, 2, 2]


def build_graph():
    nc = bacc.Bacc(None, target_bir_lowering=False, debug=False)

    hT = nc.declare_dram_parameter("hT", [HID_SH, S], F32, isOutput=False)
    rT = nc.declare_dram_parameter("rT", [HID_SH, S], F32, isOutput=False)
    pos_in = nc.declare_dram_parameter("positions", [1, S], I32, isOutput=False)
    wq_hi = nc.declare_dram_parameter("wq_hi", [HID, QKV_COLS], F8, isOutput=False)
    wq_lo = nc.declare_dram_parameter("wq_lo", [HID, QKV_COLS], F8, isOutput=False)
    wo_hi = nc.declare_dram_parameter("wo_hi", [HQ * DHEAD, HID], F8, isOutput=False)
    wo_lo = nc.declare_dram_parameter("wo_lo", [HQ * DHEAD, HID], F8, isOutput=False)
    wu_hi = nc.declare_dram_parameter("wu_hi", [HID, INT_SH], F8, isOutput=False)
    wu_lo = nc.declare_dram_parameter("wu_lo", [HID, INT_SH], F8, isOutput=False)
    wd_hi = nc.declare_dram_parameter("wd_hi", [INT_SH, HID], F8, isOutput=False)
    wd_lo = nc.declare_dram_parameter("wd_lo", [INT_SH, HID], F8, isOutput=False)
    out_res2 = nc.declare_dram_parameter("res2T", [HID_SH, S], F32, isOutput=True)
    out_mlp = nc.declare_dram_parameter("mlpT", [HID_SH, S], F32, isOutput=True)

    RG = [list(range(N_CORES))]
    inv_sqrt_d = 1.0 / math.sqrt(DHEAD)

    # weight views: row (t two p) -> [p, t(pair), two, m]
    wq_hi_v = wq_hi[:].rearrange("(t two p) m -> p t two m", two=2, p=P)
    wq_lo_v = wq_lo[:].rearrange("(t two p) m -> p t two m", two=2, p=P)
    wo_hi_v = wo_hi[:].rearrange("(t two p) m -> p t two m", two=2, p=P)
    wo_lo_v = wo_lo[:].rearrange("(t two p) m -> p t two m", two=2, p=P)
    wu_hi_v = wu_hi[:].rearrange("(t two p) m -> p t two m", two=2, p=P)
    wu_lo_v = wu_lo[:].rearrange("(t two p) m -> p t two m", two=2, p=P)
    wd_hi_v = wd_hi[:].rearrange("(t two p) m -> p t two m", two=2, p=P)
    wd_lo_v = wd_lo[:].rearrange("(t two p) m -> p t two m", two=2, p=P)

    with tile.TileContext(nc) as tc:
        with contextlib.ExitStack() as ctx:
            const = ctx.enter_context(tc.tile_pool(name="const", bufs=1))
            acc = ctx.enter_context(tc.tile_pool(name="acc", bufs=6, space="PSUM"))
            rowps = ctx.enter_context(tc.tile_pool(name="rowps", bufs=1, space="PSUM"))
            tpps = ctx.enter_context(tc.tile_pool(name="tpps", bufs=1, space="PSUM"))
            dram = ctx.enter_context(tc.tile_pool(name="dram", bufs=1, space="DRAM"))

            ones_bf = const.tile([P, 1], BF16)
            nc.vector.memset(ones_bf[:], 1.0)

            # DRAM scratch
            ag1_in = [dram.tile([BLK, SH], BF16, name=f"ag1_in{h}") for h in range(2)]
            ag1_out = [dram.tile([N_CORES * BLK, SH], BF16, name=f"ag1_out{h}",
                                 addr_space="Shared") for h in range(2)]
            ag2_in = [dram.tile([BLK, SH], BF16, name=f"ag2_in{h}") for h in range(2)]
            ag2_out = [dram.tile([N_CORES * BLK, SH], BF16, name=f"ag2_out{h}",
                                 addr_space="Shared") for h in range(2)]
            rs1_in = [dram.tile([HID, SC], BF16, name=f"rs1_in{sc}") for sc in range(NSC)]
            rs1_out = [dram.tile([HID_SH, SC], BF16, name=f"rs1_out{sc}")
                       for sc in range(NSC)]
            rs2_in = [dram.tile([mc * P, S], BF16, name=f"rs2_in{pi}")
                      for pi, mc in enumerate(PIECES)]
            rs2_out = [dram.tile([mc * P // N_CORES, S], BF16, name=f"rs2_out{pi}")
                       for pi, mc in enumerate(PIECES)]
            xbd = dram.tile([HID_SH, S], BF16, name="xbd")   # bf16 x stash

            ag1_v = [t[:].rearrange("(c r) s -> c r s", r=BLK) for t in ag1_out]
            ag2_v = [t[:].rearrange("(c r) s -> c r s", r=BLK) for t in ag2_out]
            ag1_8 = [t[:].bitcast(F8).rearrange("(c r) s -> c r s", r=BLK)
                     for t in ag1_out]
            ag2_8 = [t[:].bitcast(F8).rearrange("(c r) s -> c r s", r=BLK)
                     for t in ag2_out]

            bcd = [dram.tile([1, SC], F32, name=f"bcd{i}") for i in range(8)]
            bcdb = [dram.tile([1, SC], BF16, name=f"bcdb{i}") for i in range(8)]
            _bci = [0]

            def row_broadcast(dst_ap, src_row):
                i = _bci[0] % len(bcd)
                _bci[0] += 1
                d = bcdb[i] if dst_ap.dtype == BF16 else bcd[i]
                nc.sync.dma_start(d[:], src_row)
                nc.sync.dma_start(dst_ap, d[:].to_broadcast((P, SC)))

            # =========== era A pools (attention + residual stream) ===========
            a_es = contextlib.ExitStack()
            apers = a_es.enter_context(tc.tile_pool(name="apers", bufs=1))
            awork = a_es.enter_context(tc.tile_pool(name="awork", bufs=1))

            _cnt = [0]

            def t2k(tag, bufs):
                _cnt[0] += 1
                return awork.tile([P, SC], F32, tag=tag, bufs=bufs,
                                  name=f"t_{_cnt[0]}")

            def t1k(tag="t1k", bufs=5):
                _cnt[0] += 1
                return awork.tile([P, SC], BF16, tag=tag, bufs=bufs,
                                  name=f"t_{_cnt[0]}")

            def t8k(tag, bufs=2):
                _cnt[0] += 1
                return awork.tile([P, SC], F8, tag=tag, bufs=bufs,
                                  name=f"t_{_cnt[0]}")

            # ---- phase 1: x = h + r; hi/lo fp8 + ssq into payload ----
            with tc.tile_pool(name="p1", bufs=1) as p1:
                for sc in range(NSC):
                    cs = slice(sc * SC, (sc + 1) * SC)
                    hh = sc // 2
                    cb0 = (sc % 2) * (SC // 2)      # bf16 col offset, hi region
                    ps = rowps.tile([1, SC], F32, tag="row", name=f"ssq1p{sc}")
                    for i in range(NT_HSH):
                        a = p1.tile([P, SC], F32, tag="p1a", bufs=3, name=f"a{sc}_{i}")
                        b = p1.tile([P, SC], F32, tag="p1b", bufs=3, name=f"b{sc}_{i}")
                        nc.sync.dma_start(a[:], hT[i * P:(i + 1) * P, cs])
                        nc.sync.dma_start(b[:], rT[i * P:(i + 1) * P, cs])
                        xt = p1.tile([P, SC], F32, tag="p1x", bufs=3,
                                     name=f"x{sc}_{i}")
                        nc.vector.tensor_tensor(xt[:], a[:], b[:], ALU.add)
                        xhi = p1.tile([P, SC], F8, tag="p1hi", bufs=2,
                                      name=f"xh{sc}_{i}")
                        nc.vector.tensor_copy(xhi[:], xt[:])
                        xlo = p1.tile([P, SC], F8, tag="p1lo", bufs=2,
                                      name=f"xl{sc}_{i}")
                        nc.vector.tensor_tensor(xlo[:], xt[:], xhi[:], ALU.subtract)
                        nc.sync.dma_start(
                            ag1_in[hh][i * P:(i + 1) * P,
                                       cb0:cb0 + SC // 2].bitcast(F8), xhi[:])
                        nc.sync.dma_start(
                            ag1_in[hh][i * P:(i + 1) * P,
                                       SH // 2 + cb0:SH // 2 + cb0 + SC // 2]
                            .bitcast(F8), xlo[:])
                        sq = p1.tile([P, SC], BF16, tag="p1sq", bufs=2,
                                     name=f"sq{sc}_{i}")
                        nc.scalar.activation(sq[:], xt[:], AF.Square)
                        nc.tensor.matmul(ps[:], ones_bf[:], sq[:],
                                         start=(i == 0), stop=(i == NT_HSH - 1))
                        xb = p1.tile([P, SC], BF16, tag="p1xb", bufs=2,
                                     name=f"xb{sc}_{i}")
                        nc.scalar.activation(xb[:], xt[:], AF.Copy)
                        nc.gpsimd.dma_start(xbd[i * P:(i + 1) * P, cs], xb[:])
                    ssq1b = awork.tile([1, SC], BF16, tag="ssq1b", bufs=2,
                                       name=f"ssq1b{sc}")
                    nc.vector.tensor_copy(ssq1b[:], ps[:])
                    nc.sync.dma_start(
                        ag1_in[hh][HID_SH:HID_SH + 1,
                                   (sc % 2) * SC:(sc % 2) * SC + SC], ssq1b[:])
                    if sc % 2 == 1:
                        nc.gpsimd.collective_compute(
                            "AllGather", ALU.bypass, replica_groups=RG,
                            ins=[ag1_in[hh][:].opt()], outs=[ag1_out[hh][:].opt()])

            # ---- rope tables + masks (after AGs so phase-1 wins queues) ----
            ident = apers.tile([P, P], BF16)
            make_identity(nc, ident[:])
            cos2 = apers.tile([P, S], BF16)
            sin_neg = apers.tile([P, S], BF16)
            cmask = []
            for j in range(SC // P):
                mk = apers.tile([P, SC], BF16, name=f"cmask{j}")
                nc.vector.memset(mk[:], 1.0)
                nc.gpsimd.affine_select(mk[:], mk[:], pattern=[[1, SC]],
                                        base=-j * P, channel_multiplier=-1,
                                        compare_op=ALU.is_ge, fill=0.0)
                cmask.append(mk)

            with tc.tile_pool(name="tbl", bufs=1) as tbl:
                iot = tbl.tile([64, 1], I32)
                nc.gpsimd.iota(iot[:], pattern=[[1, 1]], base=0, channel_multiplier=1)
                iotf = tbl.tile([64, 1], F32)
                nc.vector.tensor_copy(iotf[:], iot[:])
                invf = tbl.tile([64, 1], F32)
                nc.scalar.activation(invf[:], iotf[:], AF.Exp,
                                     scale=-math.log(THETA) / 64.0)
                invf2pi = tbl.tile([64, 1], F32)
                nc.scalar.activation(invf2pi[:], invf[:], AF.Copy,
                                     scale=1.0 / TWO_PI)
                posi = tbl.tile([1, S], I32)
                nc.sync.dma_start(posi[:], pos_in[:])
                posf = tbl.tile([1, S], F32)
                nc.vector.tensor_copy(posf[:], posi[:])

                def range_reduce_sin(dst_bf, t_ap, th, negate=False):
                    n1 = tbl.tile([64, SH], I32, tag="rri", bufs=2, name="n1")
                    nc.vector.tensor_copy(n1[:], t_ap)
                    n1f = tbl.tile([64, SH], F32, tag="rrf", bufs=2, name="n1f")
                    nc.vector.tensor_copy(n1f[:], n1[:])
                    f1 = tbl.tile([64, SH], F32, tag="rrg", bufs=2, name="f1")
                    nc.vector.tensor_tensor(f1[:], t_ap, n1f[:], ALU.subtract)
                    n2 = tbl.tile([64, SH], I32, tag="rri", bufs=2, name="n2")
                    nc.vector.tensor_copy(n2[:], f1[:])
                    n2f = tbl.tile([64, SH], F32, tag="rrf", bufs=2, name="n2f")
                    nc.vector.tensor_copy(n2f[:], n2[:])
                    f2 = tbl.tile([64, SH], F32, tag="rrg", bufs=2, name="f2")
                    nc.vector.tensor_tensor(f2[:], f1[:], n2f[:], ALU.subtract)
                    nc.scalar.activation(dst_bf, f2[:], AF.Sin,
                                         scale=-TWO_PI if negate else TWO_PI)

                for th in range(2):
                    tcs = slice(th * SH, (th + 1) * SH)
                    posb = tbl.tile([64, SH], F32, tag="posb", bufs=2,
                                    name=f"posb{th}")
                    nc.gpsimd.partition_broadcast(posb[:], posf[:, tcs])
                    tfrac = tbl.tile([64, SH], F32, tag="tfr", bufs=2,
                                     name=f"tfrac{th}")
                    nc.scalar.activation(tfrac[:], posb[:], AF.Copy,
                                         scale=invf2pi[:])
                    sinb = tbl.tile([64, SH], BF16, tag="sb", bufs=2,
                                    name=f"sinb{th}")
                    sinnb = tbl.tile([64, SH], BF16, tag="snb", bufs=2,
                                     name=f"sinnb{th}")
                    range_reduce_sin(sinb[:], tfrac[:], th)
                    range_reduce_sin(sinnb[:], tfrac[:], th, negate=True)
                    tfrac2 = tbl.tile([64, SH], F32, tag="tfr2", bufs=2,
                                      name=f"tfrac2{th}")
                    nc.scalar.activation(tfrac2[:], tfrac[:], AF.Copy, bias=0.25)
                    cosb = tbl.tile([64, SH], BF16, tag="cb", bufs=2,
                                    name=f"cosb{th}")
                    range_reduce_sin(cosb[:], tfrac2[:], th)
                    nc.sync.dma_start(cos2[:64, tcs], cosb[:])
                    nc.sync.dma_start(cos2[64:, tcs], cosb[:])
                    nc.sync.dma_start(sin_neg[:64, tcs], sinnb[:])
                    nc.sync.dma_start(sin_neg[64:, tcs], sinb[:])

            # ---- persistent attention-era tiles ----
            kT = apers.tile([P, S], BF16, name="kT")
            vT = apers.tile([P, S], BF16, name="vT")
            s1b = apers.tile([P, S], BF16, name="s1b")

            # wo SBUF cache (own stack: freed after attn3)
            wo_es = contextlib.ExitStack()
            wop = wo_es.enter_context(tc.tile_pool(name="wop", bufs=1))
            wo_sb_hi = wop.tile([P, 2, 2, HID], F8, name="wo_h")
            wo_sb_lo = wop.tile([P, 2, 2, HID], F8, name="wo_l")
            nc.scalar.dma_start(wo_sb_hi[:], wo_hi_v[:])
            nc.scalar.dma_start(wo_sb_lo[:], wo_lo_v[:])

            # wq SBUF cache + gathered-x tiles (own stack: freed after qkv3)
            wq_es = contextlib.ExitStack()
            wqp = wq_es.enter_context(tc.tile_pool(name="wqp", bufs=1))
            wq_sb_hi = [wqp.tile([P, NPH, 2, P], F8, name=f"wqh{j}") for j in range(NJ)]
            wq_sb_lo = [wqp.tile([P, NPH, 2, P], F8, name=f"wql{j}") for j in range(NJ)]
            for j in range(NJ):
                nc.scalar.dma_start(wq_sb_hi[j][:],
                                    wq_hi_v[:, :, :, j * P:(j + 1) * P])
                nc.scalar.dma_start(wq_sb_lo[j][:],
                                    wq_lo_v[:, :, :, j * P:(j + 1) * P])

            def emit_s1b(sc):
                cs = slice(sc * SC, (sc + 1) * SC)
                hh = sc // 2
                hcs = slice((sc % 2) * SC, (sc % 2) * SC + SC)
                srows_b = awork.tile([8, SC], BF16, tag="srb", bufs=1,
                                     name=f"sr1b{sc}")
                nc.gpsimd.dma_start(srows_b[:], ag1_v[hh][:, HID_SH, hcs])
                srows = awork.tile([8, SC], F32, tag="srf", bufs=1,
                                   name=f"sr1f{sc}")
                nc.vector.tensor_copy(srows[:], srows_b[:])
                ssum = awork.tile([8, SC], F32, tag="ssum", bufs=1,
                                  name=f"ss1{sc}")
                nc.gpsimd.partition_all_reduce(ssum[:], srows[:], channels=8,
                                               reduce_op=ReduceOp.add)
                var = awork.tile([1, SC], F32, tag="var", bufs=2, name=f"v1{sc}")
                nc.scalar.activation(var[:], ssum[:1, :], AF.Copy,
                                     scale=1.0 / HID, bias=EPS)
                nc.vector.reciprocal(var[:], var[:])
                varb = awork.tile([1, SC], BF16, tag="varb", bufs=2,
                                  name=f"v1b{sc}")
                nc.scalar.activation(varb[:], var[:], AF.Sqrt, scale=CE1 * CE1)
                row_broadcast(s1b[:, cs], varb[:])

            qcs = {}

            def emit_qkv(sc):
                cs = slice(sc * SC, (sc + 1) * SC)
                hh = sc // 2
                c0 = (sc % 2) * SC                 # fp8 col offset, hi region
                ghi, glo = [], []
                for cb in range(N_CORES):
                    for tp in range(2):
                        g = wqp.tile([P, 2, SC], F8, tag="ghi", bufs=16,
                                     name=f"gh{cb}_{tp}_{sc}")
                        nc.gpsimd.dma_start(
                            g[:], ag1_8[hh][cb, 256 * tp:256 * (tp + 1),
                                            c0:c0 + SC]
                            .rearrange("(two p) n -> p two n", two=2))
                        ghi.append(g)
                        g = wqp.tile([P, 2, SC], F8, tag="glo", bufs=16,
                                     name=f"gl{cb}_{tp}_{sc}")
                        nc.gpsimd.dma_start(
                            g[:], ag1_8[hh][cb, 256 * tp:256 * (tp + 1),
                                            SH + c0:SH + c0 + SC]
                            .rearrange("(two p) n -> p two n", two=2))
                        glo.append(g)
                qc = {}
                for j in range(NJ):
                    ps = acc.tile([P, SC], F32, tag="acc", name=f"qk{j}_{sc}")
                    for g in range(NPH):
                        nc.tensor.matmul(ps[:], wq_sb_hi[j][:, g], ghi[g][:],
                                         start=(g == 0), stop=False, perf_mode=DR)
                    for g in range(NPH):
                        nc.tensor.matmul(ps[:], wq_sb_hi[j][:, g], glo[g][:],
                                         start=False, stop=False, perf_mode=DR)
                    for g in range(NPH):
                        nc.tensor.matmul(ps[:], wq_sb_lo[j][:, g], ghi[g][:],
                                         start=False, stop=(g == NPH - 1),
                                         perf_mode=DR)
                    if j < HQ:
                        dst = awork.tile([P, SC], BF16, tag="qc", bufs=8,
                                         name=f"qc{j}_{sc}")
                        qc[j] = dst
                        nc.vector.tensor_tensor(dst[:], ps[:], s1b[:, cs], ALU.mult)
                    else:
                        dst = kT if j == HQ else vT
                        nc.vector.tensor_tensor(dst[:, cs], ps[:], s1b[:, cs],
                                                ALU.mult)
                qcs[sc] = qc

            def emit_attn(sc):
                cs = slice(sc * SC, (sc + 1) * SC)
                qc = qcs[sc]
                # rope on q tiles + k chunk
                for j in range(HQ + 1):
                    tv = qc[j][:] if j < HQ else kT[:, cs]
                    swp = t1k()
                    nc.sync.dma_start(swp[:64, :], tv[64:, :])
                    nc.sync.dma_start(swp[64:, :], tv[:64, :])
                    m1 = t1k()
                    nc.vector.tensor_tensor(m1[:], tv, cos2[:, cs], ALU.mult)
                    m2 = t1k()
                    nc.vector.tensor_tensor(m2[:], swp[:], sin_neg[:, cs], ALU.mult)
                    nc.vector.tensor_tensor(tv, m1[:], m2[:], ALU.add)

                # v transpose in place
                for t in range(sc * (SC // P), (sc + 1) * (SC // P)):
                    pst = tpps.tile([P, P], BF16, tag="tp", name=f"tp{t}")
                    nc.tensor.transpose(pst[:], vT[:, t * P:(t + 1) * P], ident[:])
                    nc.vector.tensor_copy(vT[:, t * P:(t + 1) * P], pst[:])

                # attention: 4 heads x this chunk; fp8 hi/lo attn output
                nsk = (sc + 1) * (SC // P)
                ahi = awork.tile([P, HQ, SC], F8, tag="ahi", bufs=1,
                                 name=f"ahi{sc}")
                alo = awork.tile([P, HQ, SC], F8, tag="alo", bufs=1,
                                 name=f"alo{sc}")
                for h in range(HQ):
                    pv = acc.tile([P, SC], F32, tag="acc", name=f"pv{h}_{sc}")
                    rs = rowps.tile([1, SC], F32, tag="row", name=f"rs{h}_{sc}")
                    for skt in range(nsk):
                        sps = acc.tile([P, SC], F32, tag="acc",
                                       name=f"s{h}_{sc}_{skt}")
                        nc.tensor.matmul(sps[:], kT[:, skt * P:(skt + 1) * P],
                                         qc[h][:], start=True, stop=True)
                        ex = t1k(tag="ex", bufs=4)
                        nc.scalar.activation(ex[:], sps[:], AF.Exp,
                                             scale=inv_sqrt_d)
                        if skt >= 4 * sc:
                            nc.vector.tensor_tensor(ex[:], ex[:],
                                                    cmask[skt - 4 * sc][:],
                                                    ALU.mult)
                        nc.tensor.matmul(rs[:], ones_bf[:], ex[:],
                                         start=(skt == 0), stop=(skt == nsk - 1))
                        nc.tensor.matmul(pv[:], vT[:, skt * P:(skt + 1) * P],
                                         ex[:], start=(skt == 0),
                                         stop=(skt == nsk - 1))
                    rcp = awork.tile([1, SC], F32, tag="rcp", bufs=2,
                                     name=f"rcp{h}_{sc}")
                    nc.vector.reciprocal(rcp[:], rs[:])
                    rcpb = t2k(tag="rcpb", bufs=2)
                    row_broadcast(rcpb[:], rcp[:])
                    a32 = t2k(tag="a32", bufs=2)
                    nc.vector.tensor_tensor(a32[:], pv[:], rcpb[:], ALU.mult)
                    nc.vector.tensor_copy(ahi[:, h, :], a32[:])
                    nc.vector.tensor_tensor(alo[:, h, :], a32[:], ahi[:, h, :],
                                            ALU.subtract)

                # o_proj: 3-term DoubleRow; evict const scale -> bf16 -> RS
                for m in range(NT_HID):
                    ps = acc.tile([P, SC], F32, tag="acc", name=f"o{m}_{sc}")
                    for a2 in range(2):
                        nc.tensor.matmul(ps[:],
                                         wo_sb_hi[:, a2, :, m * P:(m + 1) * P],
                                         ahi[:, 2 * a2:2 * a2 + 2, :],
                                         start=(a2 == 0), stop=False, perf_mode=DR)
                    for a2 in range(2):
                        nc.tensor.matmul(ps[:],
                                         wo_sb_hi[:, a2, :, m * P:(m + 1) * P],
                                         alo[:, 2 * a2:2 * a2 + 2, :],
                                         start=False, stop=False, perf_mode=DR)
                    for a2 in range(2):
                        nc.tensor.matmul(ps[:],
                                         wo_sb_lo[:, a2, :, m * P:(m + 1) * P],
                                         ahi[:, 2 * a2:2 * a2 + 2, :],
                                         start=False, stop=(a2 == 1), perf_mode=DR)
                    ev = t1k(tag="oev", bufs=3)
                    nc.scalar.activation(ev[:], ps[:], AF.Copy, scale=CO)
                    nc.sync.dma_start(rs1_in[sc][m * P:(m + 1) * P, :], ev[:])
                nc.gpsimd.collective_compute(
                    "ReduceScatter", ALU.add, replica_groups=RG,
                    ins=[rs1_in[sc][:].opt()], outs=[rs1_out[sc][:].opt()])

            def emit_res2(sc):
                cs = slice(sc * SC, (sc + 1) * SC)
                hh = sc // 2
                cb0 = (sc % 2) * (SC // 2)
                ps2 = rowps.tile([1, SC], F32, tag="row", name=f"ssq2_{sc}")
                for i in range(NT_HSH):
                    o = t1k(tag="r2ld", bufs=2)
                    nc.gpsimd.dma_start(o[:], rs1_out[sc][i * P:(i + 1) * P, :])
                    xr = t1k(tag="xr", bufs=2)
                    nc.sync.dma_start(xr[:], xbd[i * P:(i + 1) * P, cs])
                    r2t = t2k(tag="r2", bufs=2)
                    nc.vector.tensor_tensor(r2t[:], o[:], xr[:], ALU.add)
                    nc.sync.dma_start(out_res2[i * P:(i + 1) * P, cs], r2t[:])
                    r2hi = t8k(tag="r2hi", bufs=2)
                    nc.vector.tensor_copy(r2hi[:], r2t[:])
                    r2lo = t8k(tag="r2lo", bufs=2)
                    nc.vector.tensor_tensor(r2lo[:], r2t[:], r2hi[:], ALU.subtract)
                    nc.sync.dma_start(
                        ag2_in[hh][i * P:(i + 1) * P,
                                   cb0:cb0 + SC // 2].bitcast(F8), r2hi[:])
                    nc.sync.dma_start(
                        ag2_in[hh][i * P:(i + 1) * P,
                                   SH // 2 + cb0:SH // 2 + cb0 + SC // 2]
                        .bitcast(F8), r2lo[:])
                    sq = t1k(tag="sq", bufs=2)
                    nc.scalar.activation(sq[:], r2t[:], AF.Square)
                    nc.tensor.matmul(ps2[:], ones_bf[:], sq[:],
                                     start=(i == 0), stop=(i == NT_HSH - 1))
                ssq2 = awork.tile([1, SC], BF16, tag="ssq2", bufs=2,
                                  name=f"sq2_{sc}")
                nc.vector.tensor_copy(ssq2[:], ps2[:])
                nc.sync.dma_start(
                    ag2_in[hh][HID_SH:HID_SH + 1,
                               (sc % 2) * SC:(sc % 2) * SC + SC], ssq2[:])

            def emit_ag2(hh):
                nc.gpsimd.collective_compute(
                    "AllGather", ALU.bypass, replica_groups=RG,
                    ins=[ag2_in[hh][:].opt()], outs=[ag2_out[hh][:].opt()])

            # ============ era A schedule ============
            emit_s1b(0)
            emit_qkv(0)
            emit_s1b(1)
            emit_qkv(1)
            emit_attn(0)
            emit_s1b(2)
            emit_qkv(2)
            emit_attn(1)
            emit_res2(0)
            emit_s1b(3)
            emit_qkv(3)
            wq_es.close()
            emit_attn(2)
            emit_res2(1)
            emit_ag2(0)
            emit_attn(3)
            wo_es.close()
            emit_res2(2)
            emit_res2(3)
            emit_ag2(1)
            a_es.close()

            # =========== era B pools (MLP) — opened early for prefetch ======
            b_es = contextlib.ExitStack()
            mpers = b_es.enter_context(tc.tile_pool(name="mpers", bufs=1))
            mwork = b_es.enter_context(tc.tile_pool(name="mwork", bufs=1))
            mstr = b_es.enter_context(tc.tile_pool(name="mstr", bufs=1))

            s2b = mpers.tile([P, S], F32, name="s2b")

            def emit_s2b(sc):
                cs = slice(sc * SC, (sc + 1) * SC)
                hh = sc // 2
                hcs = slice((sc % 2) * SC, (sc % 2) * SC + SC)
                srows_b = mwork.tile([8, SC], BF16, tag="srb2", bufs=1,
                                     name=f"sr2b{sc}")
                nc.gpsimd.dma_start(srows_b[:], ag2_v[hh][:, HID_SH, hcs])
                srows = mwork.tile([8, SC], F32, tag="srf2", bufs=1,
                                   name=f"sr2f{sc}")
                nc.vector.tensor_copy(srows[:], srows_b[:])
                ssum = mwork.tile([8, SC], F32, tag="ssum2", bufs=1,
                                  name=f"ss2{sc}")
                nc.gpsimd.partition_all_reduce(ssum[:], srows[:], channels=8,
                                               reduce_op=ReduceOp.add)
                var = mwork.tile([1, SC], F32, tag="var2", bufs=2, name=f"v2{sc}")
                nc.scalar.activation(var[:], ssum[:1, :], AF.Copy,
                                     scale=1.0 / (HID * CD), bias=EPS / CD)
                nc.vector.reciprocal(var[:], var[:])     # = CD / var
                row_broadcast(s2b[:, cs], var[:])

            def load_g2(half):
                ghi, glo = [], []
                for cb in range(N_CORES):
                    for tp in range(2):
                        g = mwork.tile([P, 2, SH], F8, tag="g2h", bufs=16,
                                       name=f"g2h{cb}_{tp}_{half}")
                        nc.gpsimd.dma_start(
                            g[:], ag2_8[half][cb, 256 * tp:256 * (tp + 1), 0:SH]
                            .rearrange("(two p) n -> p two n", two=2))
                        ghi.append(g)
                        g = mwork.tile([P, 2, SH], F8, tag="g2l", bufs=16,
                                       name=f"g2l{cb}_{tp}_{half}")
                        nc.gpsimd.dma_start(
                            g[:], ag2_8[half][cb, 256 * tp:256 * (tp + 1),
                                              SH:2 * SH]
                            .rearrange("(two p) n -> p two n", two=2))
                        glo.append(g)
                return ghi, glo

            ut_hi = [mpers.tile([P, 2, S], F8, name=f"uth{g}") for g in range(NPI)]
            ut_lo = [mpers.tile([P, 2, S], F8, name=f"utl{g}") for g in range(NPI)]

            g2 = {}

            def emit_up(half, it_range):
                ghi, glo = g2[half]
                for it in it_range:
                    wh = mstr.tile([P, NPH, 2, P], F8, tag="wuh", bufs=2,
                                   name=f"wuh{it}_{half}")
                    nc.scalar.dma_start(wh[:], wu_hi_v[:, :, :, it * P:(it + 1) * P])
                    wl = mstr.tile([P, NPH, 2, P], F8, tag="wul", bufs=2,
                                   name=f"wul{it}_{half}")
                    nc.scalar.dma_start(wl[:], wu_lo_v[:, :, :, it * P:(it + 1) * P])
                    for ci in range(2):
                        sc = 2 * half + ci
                        cs = slice(sc * SC, (sc + 1) * SC)
                        ncs = slice(ci * SC, (ci + 1) * SC)
                        ps = acc.tile([P, SC], F32, tag="acc", name=f"up{it}_{sc}")
                        for g in range(NPH):
                            nc.tensor.matmul(ps[:], wh[:, g], ghi[g][:, :, ncs],
                                             start=(g == 0), stop=False,
                                             perf_mode=DR)
                        for g in range(NPH):
                            nc.tensor.matmul(ps[:], wh[:, g], glo[g][:, :, ncs],
                                             start=False, stop=False, perf_mode=DR)
                        for g in range(NPH):
                            nc.tensor.matmul(ps[:], wl[:, g], ghi[g][:, :, ncs],
                                             start=False, stop=(g == NPH - 1),
                                             perf_mode=DR)
                        rl = mwork.tile([P, SC], F32, tag="rl", bufs=2,
                                        name=f"rl{it}_{sc}")
                        nc.scalar.activation(rl[:], ps[:], AF.Relu, scale=ALPHA_UP)
                        u32 = mwork.tile([P, SC], F32, tag="u32", bufs=2,
                                         name=f"u32{it}_{sc}")
                        nc.vector.tensor_tensor(u32[:], rl[:], rl[:], ALU.mult)
                        nc.vector.tensor_copy(ut_hi[it // 2][:, it % 2, cs], u32[:])
                        nc.vector.tensor_tensor(ut_lo[it // 2][:, it % 2, cs],
                                                u32[:], ut_hi[it // 2][:, it % 2, cs],
                                                ALU.subtract)

            # ---- era B schedule ----
            emit_s2b(0)
            emit_s2b(1)
            g2[0] = load_g2(0)
            emit_up(0, range(NT_INT))
            emit_s2b(2)
            emit_s2b(3)
            g2[1] = load_g2(1)
            emit_up(1, range(NT_INT))

            # ---- down proj: 3-term DoubleRow, uneven RS pieces ----
            mstart = 0
            for pi, mc in enumerate(PIECES):
                for mq in range(mc):
                    m = mstart + mq
                    wh = mstr.tile([P, NPI, 2, P], F8, tag="wdh", bufs=3,
                                   name=f"wdh{m}")
                    nc.scalar.dma_start(wh[:], wd_hi_v[:, :, :, m * P:(m + 1) * P])
                    wl = mstr.tile([P, NPI, 2, P], F8, tag="wdl", bufs=3,
                                   name=f"wdl{m}")
                    nc.scalar.dma_start(wl[:], wd_lo_v[:, :, :, m * P:(m + 1) * P])
                    for sc in range(NSC):
                        cs = slice(sc * SC, (sc + 1) * SC)
                        ps = acc.tile([P, SC], F32, tag="acc", name=f"dn{m}_{sc}")
                        for g in range(NPI):
                            nc.tensor.matmul(ps[:], wh[:, g], ut_hi[g][:, :, cs],
                                             start=(g == 0), stop=False,
                                             perf_mode=DR)
                        for g in range(NPI):
                            nc.tensor.matmul(ps[:], wh[:, g], ut_lo[g][:, :, cs],
                                             start=False, stop=False, perf_mode=DR)
                        for g in range(NPI):
                            nc.tensor.matmul(ps[:], wl[:, g], ut_hi[g][:, :, cs],
                                             start=False, stop=(g == NPI - 1),
                                             perf_mode=DR)
                        ev = mwork.tile([P, SC], BF16, tag="dnev", bufs=3,
                                        name=f"dev{m}_{sc}")
                        nc.vector.tensor_tensor(ev[:], ps[:], s2b[:, cs], ALU.mult)
                        nc.sync.dma_start(rs2_in[pi][mq * P:(mq + 1) * P, cs],
                                          ev[:])
                nc.gpsimd.collective_compute(
                    "ReduceScatter", ALU.add, replica_groups=RG,
                    ins=[rs2_in[pi][:].opt()], outs=[rs2_out[pi][:].opt()])
                orow = mstart * P // N_CORES
                nc.gpsimd.dma_start(
                    out_mlp[orow:orow + mc * P // N_CORES, :], rs2_out[pi][:])
                mstart += mc
            b_es.close()

    nc.compile()
    return nc


def _q8_pair(x):
    x32 = np.asarray(x, np.float32)
    hi = np.asarray(np.clip(x32, -240, 240), F8NP)
    lo = np.asarray(np.clip(x32 - hi.astype(np.float32), -240, 240), F8NP)
    return np.ascontiguousarray(hi), np.ascontiguousarray(lo)


def shard_inputs(positions, hidden_states, residual, qkv_w, o_w, up_w, down_w,
                 ln1_w, ln2_w):
    hTf = np.ascontiguousarray(np.asarray(hidden_states).reshape(S, HID).T)
    rTf = np.ascontiguousarray(np.asarray(residual).reshape(S, HID).T)
    pos = np.ascontiguousarray(np.asarray(positions).reshape(1, S))
    q_size = N_HEADS * DHEAD
    kv = N_KV * DHEAD
    w1 = np.asarray(qkv_w, np.float32) * np.asarray(ln1_w, np.float32)[:, None] * SW1
    wof = np.asarray(o_w, np.float32) * SWO
    wuf = np.asarray(up_w, np.float32) * np.asarray(ln2_w, np.float32)[:, None] * SWU
    wdf = np.asarray(down_w, np.float32) * SWD
    in_maps = []
    for c in range(N_CORES):
        wq_c = np.concatenate([
            w1[:, c * HQ * DHEAD:(c + 1) * HQ * DHEAD],
            w1[:, q_size + c * DHEAD:q_size + (c + 1) * DHEAD],
            w1[:, q_size + kv + c * DHEAD:q_size + kv + (c + 1) * DHEAD],
        ], axis=1)
        wq_h, wq_l = _q8_pair(wq_c)
        wo_h, wo_l = _q8_pair(wof[c * HQ * DHEAD:(c + 1) * HQ * DHEAD, :])
        wu_h, wu_l = _q8_pair(wuf[:, c * INT_SH:(c + 1) * INT_SH])
        wd_h, wd_l = _q8_pair(wdf[c * INT_SH:(c + 1) * INT_SH, :])
        in_maps.append({
            "hT": np.ascontiguousarray(hTf[c * HID_SH:(c + 1) * HID_SH]),
            "rT": np.ascontiguousarray(rTf[c * HID_SH:(c + 1) * HID_SH]),
            "positions": pos,
            "wq_hi": wq_h, "wq_lo": wq_l,
            "wo_hi": wo_h, "wo_lo": wo_l,
            "wu_hi": wu_h, "wu_lo": wu_l,
            "wd_hi": wd_h, "wd_lo": wd_l,
        })
    return in_maps


_CACHE = {}


def kernel(**inputs):
    from concourse.bass_utils import run_bass_kernel_spmd
    if "nc" not in _CACHE:
        _CACHE["nc"] = build_graph()
    nc = _CACHE["nc"]
    in_maps = shard_inputs(**{k: np.asarray(v) for k, v in inputs.items()})
    res = run_bass_kernel_spmd(nc, in_maps, core_ids=list(range(N_CORES)),
                               trace=False)
    res2T = np.concatenate([res.results[c]["res2T"] for c in range(N_CORES)], axis=0)
    mlpT = np.empty((HID, S), np.float32)
    for c in range(N_CORES):
        mt = res.results[c]["mlpT"]
        mstart = 0
        for pi, mc in enumerate(PIECES):
            rows = mc * P // N_CORES          # rows per core for this piece
            orow = mstart * P // N_CORES
            g0 = mstart * P + c * rows        # global hid row start
            mlpT[g0:g0 + rows] = mt[orow:orow + rows]
            mstart += mc
    mlp_out = np.ascontiguousarray(mlpT.T).reshape(1, S, HID)
    residual2 = np.ascontiguousarray(res2T.T).reshape(1, S, HID)
    return mlp_out, residual2
